# revision 38
# baseline (speedup 1.0000x reference)
"""Causal multi-head attention block (B=2, L=2048, D=1024, H=16) on 8 trn2 cores.

Sharding: core c -> batch b = c // 4, head group g = c % 4 (heads 4g..4g+4).
All matmul operands bf16 (fp32 PSUM accumulation); rel-err gate is 2e-2.

Per core:
  1. QT/KT = (W_qk x^T + b)      (d_head on partitions; 512 x 2048)
  2. V     = (x W_v^T + b_v)     (token j on partitions; [128, 256] per j-chunk)
  3. per (query-block t_, head-pair hp), J = 0..4(t_+1)-1:
     - ST for both heads into one [128,2048] PSUM tile (row-tiled: head A uses
       PE rows 0-63, head B rows 64-127), causal-narrowed on the diagonal.
       No mask matmul: the single straddling 128x128 block per diagonal chunk
       is zeroed AFTER exp by a DVE multiply with a triangular 0/1 tile.
     - one exp PER CHUNK (no-max softmax, scores ~N(0,1)) -> bf16 P tile;
       per-chunk exps keep ACT busy while the PE's ST(J+2) overwrites the
       half of the PSUM tile that exp(J) just finished reading.
     - PV: head A/B -> separate PSUM tiles at partitions 0-64, M=65
       ([V|1] -> PSUM row 64 accumulates the softmax denominator for free)
  4. per t_: den rows + O^T out of PSUM (DVE), one reciprocal for all 4
     heads; the PE half of the normalization (sel-matmul broadcast + scale)
     runs as a FILLER unit inside the next attention block so the 3.4us
     DVE reciprocal never stalls the in-order PE queue (which would let the
     HAM clock-gate re-throttle the PE to 1.2 GHz).
  5. y^T_partial = W_out,local ot_t -> bf16 DMA
Host: y[b] = sum of the 4 partials^T + b_out.
QKV / out-projection matmuls are interleaved between attention units to keep
the PE busy (HAM stays at full clock).
"""

import numpy as np
import ml_dtypes

import concourse.bass as bass
import concourse.bacc as bacc
import concourse.mybir as mybir
from concourse.tile import TileContext
from concourse.bass_utils import run_bass_kernel_spmd

B, L, D, H = 2, 2048, 1024, 16
HD = 64                      # head dim
HPC = 4                      # heads per core
DL = HPC * HD                # 256 local head dims
N_CORES = 8
SCALE = 1.0 / 8.0            # 1/sqrt(64)
FP32 = mybir.dt.float32
BF16 = mybir.dt.bfloat16
AF = mybir.ActivationFunctionType
BF = ml_dtypes.bfloat16

NKC = D // 128               # 8 contraction chunks over D
NMB = L // 512               # 4 column blocks of 512 over L
NJC = L // 128               # 16 j-chunks of 128


def build_program():
    nc = bacc.Bacc("TRN2", target_bir_lowering=False, debug=False)

    xt = nc.dram_tensor("xt", [D, L], BF16, kind="ExternalInput")
    wqk = nc.dram_tensor("wqk", [D, 2 * DL], BF16, kind="ExternalInput")
    wv = nc.dram_tensor("wv", [D, DL], BF16, kind="ExternalInput")
    wout = nc.dram_tensor("wout", [DL, D], BF16, kind="ExternalInput")
    bqk = nc.dram_tensor("bqk", [2 * DL, 1], FP32, kind="ExternalInput")
    bv = nc.dram_tensor("bv", [1, DL], FP32, kind="ExternalInput")
    trid = nc.dram_tensor("trid", [128, 128], BF16, kind="ExternalInput")
    seld = nc.dram_tensor("seld", [128, 256], BF16, kind="ExternalInput")
    yt = nc.dram_tensor("yt", [D, L], BF16, kind="ExternalOutput")

    with TileContext(nc) as tc:
        with (
            tc.tile_pool(name="const", bufs=1) as const,
            tc.tile_pool(name="xtp", bufs=32) as xtp,
            tc.tile_pool(name="ptp", bufs=6) as ptp,
            tc.tile_pool(name="rp", bufs=2) as rp,
            tc.tile_pool(name="yp", bufs=6) as yp,
            tc.tile_pool(name="ps_st", bufs=2, space="PSUM") as ps_st,
            tc.tile_pool(name="ps_ot", bufs=2, space="PSUM") as ps_ot,
            tc.tile_pool(name="ps_sm", bufs=2, space="PSUM") as ps_sm,
        ):
            # ---- HAM warm-up ----
            # Dummy matmuls on a scratch tile keep the PE busy while the
            # first DMAs land, so the clock gate opens (1.2 GHz -> 2.4 GHz)
            # before real work starts. Results are never read. The memset
            # runs on GpSimd (its engine queue starts ~1.5us earlier than
            # the DVE's at NEFF start).
            scratch = const.tile([128, 512], BF16, tag="scratch")
            nc.gpsimd.memset(scratch[:], 1.0)
            expwarm = const.tile([128, 1], BF16, tag="expwarm")
            # preload the exp ACT table set (~2.7us) during the DMA ramp
            nc.scalar.activation(expwarm[:], scratch[:, 0:1], AF.Exp)
            for _ in range(12):
                wps = ps_sm.tile([128, 512], FP32, tag="ps_sm", name="wps")
                nc.tensor.matmul(wps[:], scratch[:, 0:128], scratch[:],
                                 start=True, stop=True)

            # ---- persistent constants / weights ----
            # Three parallel DMA queues (sync=SP-HWDGE, scalar=ACT-HWDGE,
            # gpsimd), packed by first-use time. wqk columns are host-packed
            # as [q01|k01|q23|k23], so the pair-0 half (A) is the only
            # weight block the first two qk units need; x block 0 is split
            # even/odd across sync/gpsimd so it lands in ~half the time.
            wqkA_t, wqkB_t = [], []
            xts = {m: [] for m in range(NMB)}
            for kc in range(NKC):
                t = const.tile([128, DL], BF16, tag=f"wqkA{kc}")
                nc.scalar.dma_start(out=t[:],
                                    in_=wqk[kc * 128:(kc + 1) * 128, 0:DL])
                wqkA_t.append(t)
                tx = xtp.tile([128, 512], BF16, name="t")
                eng = nc.sync if kc % 2 == 0 else nc.gpsimd
                eng.dma_start(
                    out=tx[:], in_=xt[kc * 128:(kc + 1) * 128, 0:512])
                xts[0].append(tx)
            bq_t = []
            for nt in range(4):
                t = const.tile([128, 1], FP32, tag=f"bqk{nt}")
                nc.gpsimd.dma_start(out=t[:],
                                    in_=bqk[nt * 128:(nt + 1) * 128, :])
                bq_t.append(t)

            def load_consts2():
                wv_t = []
                for kc in range(NKC):
                    t = const.tile([128, DL], BF16, tag=f"wv{kc}")
                    nc.scalar.dma_start(out=t[:],
                                        in_=wv[kc * 128:(kc + 1) * 128, :])
                    wv_t.append(t)
                bvrep = const.tile([128, DL], FP32, tag="bvrep")
                nc.gpsimd.dma_start(out=bvrep[:],
                                    in_=bv[0:1, :].to_broadcast((128, DL)))
                tri_t = const.tile([128, 128], BF16, tag="tri")
                nc.gpsimd.dma_start(out=tri_t[:], in_=trid[:, :])
                # pair-1 weight half (B) after wv: first needed ~2 slots
                # into block 0's filler stream
                for kc in range(NKC):
                    t = const.tile([128, DL], BF16, tag=f"wqkB{kc}")
                    nc.scalar.dma_start(
                        out=t[:],
                        in_=wqk[kc * 128:(kc + 1) * 128, DL:2 * DL])
                    wqkB_t.append(t)
                sel_t = const.tile([128, 256], BF16, tag="sel")
                nc.gpsimd.dma_start(out=sel_t[:], in_=seld[:, :])
                return wv_t, bvrep, tri_t, sel_t

            def load_consts3():
                wout_t = []
                for n2 in range(2):
                    t = const.tile([128, D], BF16, tag=f"wout{n2}")
                    nc.scalar.dma_start(out=t[:],
                                        in_=wout[n2 * 128:(n2 + 1) * 128, :])
                    wout_t.append(t)
                return wout_t

            # persistent activations
            # qk_t[0..1]: QT tiles (128 rows: heads {2i,2i+1}); qk_t[2..3]: KT
            qk_t = [const.tile([128, L], BF16, tag=f"qk{nt}", name=f"qk{nt}")
                    for nt in range(4)]
            # V tiles per j-chunk: [128, 4*65]; head h cols h*65..h*65+64 = V,
            # col h*65+64 = 1.0 (accumulates the softmax denominator as PSUM
            # row 64 of the PV output, for free)
            v_t = [const.tile([128, 4 * 65], BF16, tag=f"v{j}", name=f"v{j}")
                   for j in range(NJC)]
            ot_t = [const.tile([128, L], BF16, tag=f"ot{n2}", name=f"ot{n2}")
                    for n2 in range(2)]

            def load_x(m):
                for kc in range(NKC):
                    t = xtp.tile([128, 512], BF16, name="t")
                    eng = nc.sync if kc % 2 == 0 else nc.gpsimd
                    eng.dma_start(
                        out=t[:],
                        in_=xt[kc * 128:(kc + 1) * 128, m * 512:(m + 1) * 512])
                    xts[m].append(t)

            def keep_warm(ps, n):
                # dummy matmuls into the unit's own (about-to-be-overwritten)
                # PSUM tile: they fill the PE while this DMA-paced unit's
                # inputs land, so the HAM clock gate never sees an idle
                # window and re-throttles the PE to 1.2 GHz. The real chain
                # starts with start=True, clobbering whatever these wrote.
                for _ in range(n):
                    nc.tensor.matmul(ps[:], scratch[:, 0:128], scratch[:],
                                     start=True, stop=True)

            def qk_unit(m, nt, warm=0):
                # nt: 0 = pair-0 QT, 1 = pair-0 KT, 2 = pair-1 QT,
                # 3 = pair-1 KT (matches the host wqk column packing)
                wt = wqkA_t if nt < 2 else wqkB_t
                c0 = (nt % 2) * 128
                ps = ps_sm.tile([128, 512], FP32, tag="ps_sm")
                keep_warm(ps, warm)
                for kc in range(NKC):
                    nc.tensor.matmul(
                        ps[:],
                        wt[kc][:, c0:c0 + 128],
                        xts[m][kc][:],
                        start=(kc == 0), stop=(kc == NKC - 1))
                with nc.allow_low_precision(reason="bf16 activations"):
                    nc.vector.tensor_scalar_add(
                        qk_t[nt][:, m * 512:(m + 1) * 512], ps[:], bq_t[nt][:])

            def v_unit(m, ic, warm=0):
                j = 4 * m + ic
                ps = ps_sm.tile([128, 512], FP32, tag="ps_sm")
                keep_warm(ps, warm)
                for kc in range(NKC):
                    nc.tensor.matmul(
                        ps[:, 0:DL],
                        xts[m][kc][:, ic * 128:(ic + 1) * 128],
                        wv_t[kc][:],
                        start=(kc == 0), stop=(kc == NKC - 1))
                v4 = v_t[j][:].rearrange("p (h m) -> p h m", m=65)
                with nc.allow_low_precision(reason="bf16 activations"):
                    nc.vector.tensor_add(
                        v4[:, :, 0:64],
                        ps[:, 0:DL].rearrange("p (h d) -> p h d", d=64),
                        bvrep[:].rearrange("p (h d) -> p h d", d=64))
                nc.vector.memset(v4[:, :, 64:65], 1.0)

            def attn_pair(hp, t_, ls4, filler):
                """One head pair's ST -> exp -> PV chain over all j-chunks.
                One exp per chunk ([128,1024] covering both heads) so the PE
                can refill the other half of `big` while ACT streams; the
                diagonal straddling 128x128 block is zeroed post-exp by a
                DVE triangular multiply (no mask matmuls)."""
                n_j = 4 * (t_ + 1)
                qt = qk_t[2 * hp]
                kt = qk_t[2 * hp + 1]
                otps = [ps_ot.tile([128, 512], FP32, tag="ps_ot",
                                   name=f"otp{i}") for i in range(2)]
                # per-chunk score tiles from a 2-deep pool: Tile's WAR
                # tracking is tile-granular, so a shared big tile would make
                # every ST wait on the PREVIOUS chunk's exp read. With a
                # fresh [128,1024] tile per chunk, ST(J+2) only waits on
                # exp(J) (its buffer's last reader) and hides under exp(J+1).
                big_t = {}

                def do_st(J):
                    q = J - 4 * t_      # >= 0 on the diagonal band
                    w0 = 128 * q if q > 0 else 0
                    big = ps_st.tile([128, 1024], FP32, tag="ps_st",
                                     name="big")
                    big_t[J] = big
                    for i in range(2):
                        po = i * 64
                        ssl = slice(i * 512 + w0, (i + 1) * 512)
                        nc.tensor.matmul(
                            big[:, ssl],
                            kt[po:po + 64, J * 128:(J + 1) * 128],
                            qt[po:po + 64, t_ * 512 + w0:(t_ + 1) * 512],
                            start=True, stop=True)

                def do_exp(J):
                    """exp for one chunk -> [128,1024] P tile (both heads)."""
                    q = J - 4 * t_
                    big = big_t.pop(J)
                    ptile = ptp.tile([128, 1024], BF16, name="pt")
                    if q >= 0:
                        w0 = 128 * q if q > 0 else 0
                        src = big[:].rearrange(
                            "p (c n) -> p c n", n=512)[:, :, w0:512]
                        dst = ptile[:].rearrange(
                            "p (c n) -> p c n", n=512)[:, :, w0:512]
                        nc.scalar.activation(dst, src, AF.Exp, scale=SCALE)
                        # zero the invalid triangle of the straddling block
                        with nc.allow_low_precision(reason="bf16 mask"):
                            for i in range(2):
                                msl = slice(i * 512 + w0, i * 512 + w0 + 128)
                                nc.vector.tensor_mul(
                                    ptile[:, msl], ptile[:, msl], tri_t[:])
                    else:
                        nc.scalar.activation(ptile[:], big[:],
                                             AF.Exp, scale=SCALE)
                    return ptile

                def do_pv(J, ptile):
                    q = J - 4 * t_
                    w0 = 128 * q if q > 0 else 0
                    for i in range(2):
                        h65 = (2 * hp + i) % 4 * 65
                        nc.tensor.matmul(
                            otps[i][0:65, w0:512],
                            v_t[J][:, h65:h65 + 65],
                            ptile[:, i * 512 + w0:(i + 1) * 512],
                            start=(J == 0), stop=(J == n_j - 1),
                            skip_group_check=True)

                do_st(0)
                do_st(1)
                for J in range(n_j):
                    ptile = do_exp(J)
                    if J + 2 < n_j:
                        do_st(J + 2)
                    do_pv(J, ptile)
                    filler()
                # copy the denominator rows first (they gate the reciprocal),
                # then O^T|pair, out of PSUM
                osb = rp.tile([128, 512], FP32, name="osb", tag=f"osb{hp}")
                for i in range(2):
                    nc.vector.tensor_copy(ls4[32 * (2 * hp + i):
                                              32 * (2 * hp + i) + 1, :],
                                          otps[i][64:65, :])
                for i in range(2):
                    nc.vector.tensor_copy(osb[64 * i:64 * i + 64, :],
                                          otps[i][0:64, :])
                return osb

            def norm_pe(osbs, linv, t_):
                """PE half of the normalization: sel-matmul broadcast of the
                reciprocals, elementwise scale into ot_t. Runs as a filler
                unit of the NEXT block (linv comes from the DVE reciprocal
                issued at the previous block's boundary)."""
                isl = slice(t_ * 512, (t_ + 1) * 512)
                for n2 in range(2):
                    rb = ps_sm.tile([128, 512], FP32, tag="ps_sm")
                    nc.tensor.matmul(rb[:],
                                     sel_t[:, n2 * 128:(n2 + 1) * 128],
                                     linv[:], start=True, stop=True)
                    with nc.allow_low_precision(reason="bf16 activations"):
                        nc.vector.tensor_mul(ot_t[n2][:, isl], osbs[n2][:],
                                             rb[:])

            def proj_unit(t_, dt_, tail=False):
                isl = slice(t_ * 512, (t_ + 1) * 512)
                if tail:
                    # attention is done: rotate across all free PSUM pools so
                    # the last 8 projection units pipeline 4-deep
                    pool, tag = ((ps_sm, "ps_sm"), (ps_ot, "ps_ot"))[dt_ % 2]
                    ps = pool.tile([128, 512], FP32, tag=tag, name="ps")
                else:
                    ps = ps_sm.tile([128, 512], FP32, tag="ps_sm")
                for n2 in range(2):
                    nc.tensor.matmul(
                        ps[:],
                        wout_t[n2][:, dt_ * 128:(dt_ + 1) * 128],
                        ot_t[n2][:, isl],
                        start=(n2 == 0), stop=(n2 == 1))
                ys = yp.tile([128, 512], BF16, name="ys")
                with nc.allow_low_precision(reason="bf16 output"):
                    if tail:
                        # the scalar engine is idle at the kernel tail
                        nc.scalar.copy(ys[:], ps[:])
                    else:
                        nc.vector.tensor_copy(ys[:], ps[:])
                # alternate output queues so the final drain overlaps
                eng = nc.sync if dt_ % 2 == 0 else nc.gpsimd
                eng.dma_start(
                    out=yt[dt_ * 128:(dt_ + 1) * 128, isl], in_=ys[:])

            # ---- program ----
            wv_t, bvrep, tri_t, sel_t = load_consts2()
            wout_t = load_consts3()
            for m in (1, 2, 3):
                load_x(m)
            # only what t_=0 pair 0 needs up front: QT/KT heads 0-1 + first
            # two V chunks; the rest of block 0 becomes t_=0 filler work.
            # warm= pads DMA-paced units with keep-warm dummy matmuls.
            qk_unit(0, 0)
            qk_unit(0, 1)
            v_unit(0, 0)
            v_unit(0, 1)

            # attention block order (0, 1, 3, 2): t_=0 starts right after
            # QKV block 0; its fillers compute the rest of m=0 + start m=1;
            # t_=1 finishes m=1 and computes m=3 (needed by block t_=3);
            # t_=3 gets m=2 + norm(1) + proj(0)/proj(1); t_=2 (last) gets
            # norm(3) + proj(3); norm(2) + proj(2) trail at the end.
            # Each plan entry is (deadline_slot, unit): the unit must be
            # ISSUED by the end of that filler slot (1-based; block t_ has
            # 8*(t_+1) slots, one per chunk, pair 1 starting at 4*(t_+1)+1).
            # 999 = no deadline (unit only feeds a later block; the end-of-
            # block flush guarantees it still issues inside this block).
            # Deadlines come from: ST(J+2) is issued in iteration J, PV(J)
            # in iteration J, and qt/kt columns must exist before the pair
            # that reads them starts.
            filler_plan = {0: [], 1: [], 3: [], 2: []}
            filler_plan[0] += [
                (2, lambda: v_unit(0, 2)),     # PV(2) pair 0
                (3, lambda: v_unit(0, 3)),     # PV(3) pair 0
                (4, lambda: qk_unit(0, 2)),    # pair 1 qt
                (4, lambda: qk_unit(0, 3)),    # pair 1 kt
                (999, lambda: qk_unit(1, 0)),  # block 1 pair-0 qt
                (999, lambda: v_unit(1, 0)),
                (999, lambda: qk_unit(1, 1)),  # block 1 pair-0 kt m=1
                (999, lambda: v_unit(1, 1)),
                # x1-fed units pulled forward: block 0's pair-1 stretch is
                # DMA-paced, and without real filler here the PE idles long
                # enough for the HAM clock gate to re-throttle it
                (999, lambda: qk_unit(1, 2)),  # block 1 pair-1 qt
                (999, lambda: v_unit(1, 2)),
            ]
            # block 1: finish m=1, then the two m=3 QT units (block 3's qt);
            # the rest of m=3 moves into block 3 itself (its KT m=3 columns
            # and v_t[12..15] aren't read until iterations 10+)
            filler_plan[1] += [
                (7, lambda: v_unit(1, 3)),     # PV(7) pair 0
                (10, lambda: qk_unit(1, 3)),   # kt m=1, ST(4) pair 1
                (999, lambda: qk_unit(3, 0)),  # block 3 pair-0 qt
                (999, lambda: qk_unit(3, 2)),  # block 3 pair-1 qt
                # block-2 QT units backfilled from block 3 (x2 lands ~25us)
                (999, lambda: qk_unit(2, 0)),  # block 2 pair-0 qt
                (999, lambda: qk_unit(2, 2)),  # block 2 pair-1 qt
            ]
            # block 3: its own KT m=2 / m=3 columns and v_t[8..15] have
            # in-block deadlines; m=2 QT/V (block 2's inputs) and proj(0,·)
            # ride along without deadlines
            filler_plan[3] += [
                (6, lambda: qk_unit(2, 1)),    # kt m=2, ST(8) pair 0
                (8, lambda: v_unit(2, 0)),     # PV(8) pair 0
                (9, lambda: v_unit(2, 1)),
                (10, lambda: qk_unit(3, 1)),   # kt m=3, ST(12) pair 0
                (10, lambda: v_unit(2, 2)),
                (11, lambda: v_unit(2, 3)),
                (12, lambda: v_unit(3, 0)),    # PV(12) pair 0
                (13, lambda: v_unit(3, 1)),
                (14, lambda: v_unit(3, 2)),
                (15, lambda: v_unit(3, 3)),
                (22, lambda: qk_unit(2, 3)),   # kt m=2, ST(8) pair 1
                (26, lambda: qk_unit(3, 3)),   # kt m=3, ST(12) pair 1
            ]
            for dt_ in range(4):
                filler_plan[3].append(
                    (999, lambda dt_=dt_: proj_unit(0, dt_)))
            # block 2 (last) gets the bulk of the proj fillers: its exp
            # chain has the largest PE deficit and starving it lets the HAM
            # clock-gate re-throttle the PE for the whole kernel tail
            for dt_ in range(4, 8):
                filler_plan[2].append(
                    (999, lambda dt_=dt_: proj_unit(0, dt_)))
            for dt_ in range(8):
                filler_plan[2].append(
                    (999, lambda dt_=dt_: proj_unit(1, dt_)))
            for dt_ in range(4):
                filler_plan[2].append(
                    (999, lambda dt_=dt_: proj_unit(3, dt_)))
            # held OUT of the spread: issued at the tail so the PE has ~2us
            # of work queued while the last block's reciprocal chain (den
            # copies -> ln -> table load -> exp) resolves, which also keeps
            # the HAM clock gate open for the final projection units
            tail_reserve = [lambda dt_=dt_: proj_unit(3, dt_)
                            for dt_ in range(4, 8)]

            pending = []                 # (pos, unit) from the previous block
            for t_ in (0, 1, 3, 2):
                units = filler_plan[t_]
                # the previous block's reciprocal (3.4us of DVE) runs as the
                # second unit of THIS block — late enough that the first
                # unit's DVE tail precedes it in the queue, early enough
                # that norm_pe (a few units later) finds linv ready
                for pos, u in pending:
                    units.insert(min(pos, len(units)), (999, u))
                pending = []
                n_slots = 8 * (t_ + 1)   # filler call sites this block
                state = {"i": 0, "slot": 0}

                def filler(state=state, units=units, n_slots=n_slots):
                    # even pace, overridden by per-unit issue deadlines
                    state["slot"] += 1
                    want = (len(units) * state["slot"] + n_slots - 1) // n_slots
                    for i, (dl, _) in enumerate(units):
                        if dl <= state["slot"]:
                            want = max(want, i + 1)
                    while state["i"] < min(want, len(units)):
                        units[state["i"]][1]()
                        state["i"] += 1

                ls4 = rp.tile([128, 512], FP32, name="ls4", tag="ls4")
                nc.vector.memset(ls4[:], 1.0)
                osbs = []
                for hp in range(2):
                    osbs.append(attn_pair(hp, t_, ls4, filler))
                # flush leftover fillers BEFORE the reciprocal so their PE
                # work is already queued when the DVE starts the 3.4us recip
                while state["i"] < len(units):
                    units[state["i"]][1]()
                    state["i"] += 1
                linv = rp.tile([128, 512], BF16, name="linv", tag="linv")
                if t_ == 2:
                    # last block: run the reciprocal on the (by now idle)
                    # ACT engine as exp(-ln(den)) — 2 x 0.7us there beats
                    # the 3.4us DVE iterative-divide on the tail critical
                    # path (den > 0 always; ln in fp32, so the exp sees
                    # full-precision input). The reserved proj units keep
                    # the PE fed while the chain resolves.
                    with nc.allow_low_precision(reason="bf16 norm scale"):
                        lntmp = rp.tile([128, 512], FP32, name="lntmp",
                                        tag="lntmp")
                        nc.scalar.activation(lntmp[:], ls4[:], AF.Ln)
                        nc.scalar.activation(linv[:], lntmp[:], AF.Exp,
                                             scale=-1.0)
                    for u in tail_reserve:
                        u()
                    norm_pe(osbs, linv, t_)
                else:
                    def recip_unit(linv=linv, ls4=ls4):
                        with nc.allow_low_precision(reason="bf16 norm"):
                            nc.vector.reciprocal(linv[:], ls4[:])
                    pending = [
                        (1, recip_unit),
                        (5, lambda osbs=osbs, linv=linv, t_=t_:
                            norm_pe(osbs, linv, t_)),
                    ]
            for dt_ in range(8):
                proj_unit(2, dt_, tail=True)

    nc.compile()
    return nc


_NC_CACHE = None


def _get_nc():
    global _NC_CACHE
    if _NC_CACHE is None:
        _NC_CACHE = build_program()
    return _NC_CACHE


def make_in_maps(x, W_qkv, b_qkv, W_out):
    """Per-core input dicts (core c -> batch c//4, head group c%4)."""
    jj = np.arange(128)[:, None]
    qq = np.arange(128)[None, :]
    trid = (qq >= jj).astype(BF)     # valid (unmasked) iff query >= key
    seld = np.zeros((128, 256), BF)
    seld[0, 0:64] = 1
    seld[32, 64:128] = 1
    seld[64, 128:192] = 1
    seld[96, 192:256] = 1

    in_maps = []
    for c in range(N_CORES):
        b, g = divmod(c, 4)
        rs = slice(DL * g, DL * g + DL)
        wq = W_qkv[0 * D:1 * D][rs]
        wk = W_qkv[1 * D:2 * D][rs]
        wvl = W_qkv[2 * D:3 * D][rs]
        bq = b_qkv[0 * D:1 * D][rs]
        bk = b_qkv[1 * D:2 * D][rs]
        in_maps.append({
            "xt": np.ascontiguousarray(x[b].T).astype(BF),
            # column packing [q01 | k01 | q23 | k23]: the pair-0 half is
            # all the first two qk units need, so it can DMA first
            "wqk": np.ascontiguousarray(
                np.concatenate([wq[0:128], wk[0:128],
                                wq[128:256], wk[128:256]], 0).T).astype(BF),
            "wv": np.ascontiguousarray(wvl.T).astype(BF),
            "wout": np.ascontiguousarray(W_out[:, rs].T).astype(BF),
            "bqk": np.ascontiguousarray(
                np.concatenate([bq[0:128], bk[0:128],
                                bq[128:256], bk[128:256]])[:, None],
                np.float32),
            "bv": np.ascontiguousarray(b_qkv[2 * D:3 * D][rs][None, :],
                                       np.float32),
            "trid": trid,
            "seld": seld,
        })
    return in_maps


def assemble_output(results, b_out):
    y = np.zeros((B, L, D), np.float32)
    for c in range(N_CORES):
        b = c // 4
        y[b] += results[c]["yt"].T.astype(np.float32)
    y += b_out[None, None, :].astype(np.float32)
    return y


def run(x, mask, W_qkv, b_qkv, W_out, b_out, trace=False, **spmd_kwargs):
    causal = np.array_equal(
        np.asarray(mask).reshape(L, L),
        np.triu(np.ones((L, L), bool), k=1))
    if not causal:
        # Fallback (never expected): reference semantics on host.
        print("WARNING: non-causal mask; computing on host")
        q, k, v = np.split(x @ W_qkv.T + b_qkv, 3, axis=-1)
        th = lambda t: t.reshape(B, L, H, HD).transpose(0, 2, 1, 3)
        q, k, v = th(q), th(k), th(v)
        a = np.einsum('bhqd,bhkd->bhqk', q, k) * SCALE
        a = np.where(np.asarray(mask), -np.inf, a)
        a = a - a.max(-1, keepdims=True)
        a = np.exp(a)
        a /= a.sum(-1, keepdims=True)
        o = np.einsum('bhqk,bhkd->bhqd', a, v)
        o = o.transpose(0, 2, 1, 3).reshape(B, L, D)
        return o @ W_out.T + b_out, None

    nc = _get_nc()
    in_maps = make_in_maps(np.asarray(x), np.asarray(W_qkv),
                           np.asarray(b_qkv), np.asarray(W_out))
    res = run_bass_kernel_spmd(nc, in_maps, list(range(N_CORES)),
                               trace=trace, **spmd_kwargs)
    y = assemble_output(res.results, np.asarray(b_out))
    return y, res


def kernel(x, mask, W_qkv, b_qkv, W_out, b_out):
    y, _ = run(x, mask, W_qkv, b_qkv, W_out, b_out)
    return y
